# revision 1
# baseline (speedup 1.0000x reference)
"""Multi-head causal attention (B=4, N=2048, D=1024, H=16) on 8 NeuronCores.

Sharding: data-parallel over batch (4) x tensor-parallel over heads (2 halves
of 8 heads each), Megatron-style.  Core c handles batch c//2 and head-half
c%2: it computes Q/K/V projections restricted to its 512 output dims, full
causal attention for its 8 heads, and a partial output projection
out_partial = ao @ Wo[:, cols].T.  The host sums the two partials per batch
(the "all-reduce") when gathering.

Device kernel (per core):
  - x transposed on PE (128x128 transpose-mode matmuls) into xT [c, n]
  - Q^T, K^T [dh, n] and V [n, dh] via float32r matmuls
  - S^T[k,q] = K Q^T per head, k-tile (128) x q-chunk (512), causal tile skip;
    2 heads issued back-to-back at partition bases 0/64 (PE row packing)
  - causal mask on diagonal tiles added in-PSUM via identity-matmul
  - softmax: exp on ScalarE from PSUM (scale=1/8 folded in); row sums via a
    ones-column appended to V (AV matmul M=65); normalization folded into the
    PSUM->SBUF copy of the attention output
  - out-projection interleaved with attention per q-chunk

`reps` replays the whole computation N times (for slope timing).
`phase` builds probe variants: "all", "p1", "attn", "exp0", "noav".
"""

import numpy as np

import concourse.bass as bass
import concourse.bacc as bacc
import concourse.mybir as mybir
import concourse.tile as tile
from concourse.bass_utils import run_bass_kernel_spmd

B, N, D, H = 4, 2048, 1024, 16
HD = 64          # head dim
DH = 512         # per-core slice of D (8 heads)
NT = N // 128    # 16 seq tiles
CT = D // 128    # 8 feature tiles
NEG = -1e30

f32 = mybir.dt.float32
f32r = mybir.dt.float32r
EXP = mybir.ActivationFunctionType.Exp


def _junk3(src_dram, rows=1024):
    """View [rows,1024] f32 dram as [128, rows//128, 1024] f32r junk."""
    return (src_dram[0:rows, :].bitcast(f32r)
            .rearrange("(o p) f -> p o f", p=128))


def _phase1(nc, tc, rep, h, phase):
    """Transposes + Q/K/V projections into h["QT"]/h["KT"]/h["Vp"]."""
    QT, KT, Vp = h["QT"], h["KT"], h["Vp"]
    with (
        tc.tile_pool(name=f"wpool{rep}", bufs=2) as wpool,
        tc.tile_pool(name=f"xtile{rep}", bufs=3) as xtile,
        tc.tile_pool(name=f"xT{rep}", bufs=2) as xT_pool,
        tc.tile_pool(name=f"ps_t{rep}", bufs=4, space="PSUM") as ps_t,
        tc.tile_pool(name=f"ps_proj{rep}", bufs=4, space="PSUM") as ps_proj,
    ):
        # ones columns of Vp (projection writes cols 0:64 of each 65-block)
        ones_f32 = wpool.tile([128, NT, 8, 1], f32, tag="ones")
        nc.vector.memset(ones_f32[:], 1.0)
        nc.vector.tensor_copy(Vp[:, :, :, 64:65], ones_f32[:])

        for name, x_in, w_in in (
            ("k", h["xk"], h["wkT"]),
            ("v", h["xv"], h["wvT"]),
            ("q", h["xq"], h["wqT"]),
        ):
            w_t = wpool.tile([128, CT, DH], f32r, tag="w")
            nc.sync.dma_start(
                w_t[:],
                w_in[:].bitcast(f32r).rearrange("(o p) f -> p o f", p=128),
            )
            for sc in range(4):  # seq chunks of 512
                xTc = xT_pool.tile([128, CT, 512], f32r, tag="xT")
                for st in range(4):  # 128-row tiles within the chunk
                    x_t = xtile.tile([128, D], f32, tag="xt")
                    row0 = sc * 512 + st * 128
                    nc.sync.dma_start(x_t[:], x_in[row0:row0 + 128, :])
                    for ct in range(CT):
                        pst = ps_t.tile([128, 128], f32, tag="pst")
                        nc.tensor.transpose(
                            pst[:], x_t[:, ct * 128:(ct + 1) * 128],
                            h["ident_t"][:],
                        )
                        dst = xTc[:, ct, st * 128:(st + 1) * 128]
                        if (st + ct) % 2:
                            nc.vector.tensor_copy(dst, pst[:])
                        else:
                            nc.scalar.copy(dst, pst[:])
                if name in ("k", "q"):
                    dstT = KT if name == "k" else QT
                    for dt_ in range(4):
                        ps = ps_proj.tile([128, 512], f32, tag="pp")
                        for ct in range(CT):
                            nc.tensor.matmul(
                                ps[:],
                                lhsT=w_t[:, ct, dt_ * 128:(dt_ + 1) * 128],
                                rhs=xTc[:, ct, :],
                                start=(ct == 0), stop=(ct == CT - 1),
                            )
                        dst = dstT[:, dt_, sc * 512:(sc + 1) * 512]
                        if dt_ % 2:
                            nc.vector.tensor_copy(dst, ps[:])
                        else:
                            nc.scalar.copy(dst, ps[:])
                else:  # v: natural layout [n, dh], strided into Vp 65-blocks
                    for st in range(4):
                        ps = ps_proj.tile([128, 512], f32, tag="pp")
                        for ct in range(CT):
                            nc.tensor.matmul(
                                ps[:],
                                lhsT=xTc[:, ct, st * 128:(st + 1) * 128],
                                rhs=w_t[:, ct, :],
                                start=(ct == 0), stop=(ct == CT - 1),
                            )
                        kt_idx = sc * 4 + st
                        src = ps[:].rearrange("p (h d) -> p h d", h=8)
                        dst = Vp[:, kt_idx, :, 0:64]
                        if st % 2:
                            nc.vector.tensor_copy(dst, src)
                        else:
                            nc.scalar.copy(dst, src)


def _phase2(nc, tc, rep, h, phase):
    """Attention + out-projection, per q-chunk."""
    QT, KT, Vp = h["QT"], h["KT"], h["Vp"]
    out, woT_t = h["out"], h["woT_t"]
    with (
        tc.tile_pool(name=f"ao{rep}", bufs=1) as ao_pool,
        tc.tile_pool(name=f"ps_s{rep}", bufs=(3 if phase == "deep" else 2),
                     space="PSUM") as ps_s,
        tc.tile_pool(name=f"ps_av{rep}", bufs=2, space="PSUM") as ps_av,
        tc.tile_pool(name=f"pP{rep}", bufs=4) as pP,
        tc.tile_pool(name=f"pout{rep}", bufs=3) as pout,
        tc.tile_pool(name=f"small{rep}", bufs=4) as small,
    ):
        aoT = ao_pool.tile([128, 4, N], f32r, name="aoT")
        if phase in ("exp0", "noav"):
            # junk-fill buffers that skipped stages would have written
            nc.sync.dma_start(
                aoT[:].rearrange("p a b -> p (a b)")
                .rearrange("p (o f) -> p o f", o=8),
                _junk3(h["xq"]),
            )
            for pw in range(3):
                pwt = pP.tile([128, 2, 512], f32r, tag="p", name=f"pw{pw}")
                nc.sync.dma_start(
                    pwt[:].rearrange("p a b -> p (a b)"),
                    h["xk"][pw * 128:pw * 128 + 128, :].bitcast(f32r),
                )

        for qc in range(4):
            ktmax = qc * 4 + 4
            q0, q1 = qc * 512, (qc + 1) * 512
            for pr in range(4):  # head pairs
                if phase != "noav":
                    av = [
                        ps_av.tile([65, 512], f32, tag="av", name="av0"),
                        ps_av.tile([65, 512], f32, tag="av", name="av1"),
                    ]
                for kb in range(0, ktmax, 2):
                    nkt = min(2, ktmax - kb)
                    s_ps = [
                        ps_s.tile([128, 2, 512], f32, tag="s", name="s0"),
                        ps_s.tile([128, 2, 512], f32, tag="s", name="s1"),
                    ]
                    for kti in range(nkt):
                        kt = kb + kti
                        diag = kt >= qc * 4 and phase != "nomask"
                        for h2 in (0, 1):
                            p0, p1 = h2 * 64, h2 * 64 + 64
                            nc.tensor.matmul(
                                s_ps[h2][:, kti, :],
                                lhsT=KT[p0:p1, pr, kt * 128:(kt + 1) * 128],
                                rhs=QT[p0:p1, pr, q0:q1],
                                start=True, stop=not diag,
                            )
                        if diag:
                            jj = kt - qc * 4
                            msl = h["maskB_t"][:, 384 - jj * 128:896 - jj * 128]
                            for h2 in (0, 1):
                                nc.tensor.matmul(
                                    s_ps[h2][:, kti, :],
                                    lhsT=h["ident_r"][:],
                                    rhs=msl,
                                    start=False, stop=True,
                                )
                    p_sb = [
                        pP.tile([128, 2, 512], f32r, tag="p", name="p0"),
                        pP.tile([128, 2, 512], f32r, tag="p", name="p1"),
                    ]
                    for h2 in (0, 1):
                        if phase == "exp0":
                            nc.scalar.activation(
                                p_sb[h2][:, :nkt, 0:1],
                                s_ps[h2][:, :nkt, 0:1],
                                EXP, scale=0.125,
                            )
                        else:
                            nc.scalar.activation(
                                p_sb[h2][:, :nkt, :], s_ps[h2][:, :nkt, :],
                                EXP, scale=0.125,
                            )
                    if phase != "noav":
                        for kti in range(nkt):
                            kt = kb + kti
                            for h2 in (0, 1):
                                hh = pr * 2 + h2
                                nc.tensor.matmul(
                                    av[h2][:],
                                    lhsT=Vp[:, kt, hh, :],
                                    rhs=p_sb[h2][:, kti, :],
                                    start=(kt == 0), stop=(kt == ktmax - 1),
                                )
                if phase != "noav":
                    # normalize: aoT[head rows, chunk] = av[0:64] / sums.
                    # Copy PSUM->SBUF first so the AV bank frees quickly.
                    for h2 in (0, 1):
                        av_sb = small.tile([65, 512], f32, tag="avsb",
                                           name="av_sb")
                        nc.vector.tensor_copy(av_sb[:], av[h2][:])
                        r_t = small.tile([1, 512], f32, tag="r", name="r_t")
                        nc.vector.reciprocal(r_t[:], av_sb[64:65, :])
                        R_t = small.tile([64, 512], f32, tag="R", name="R_t")
                        nc.gpsimd.partition_broadcast(R_t[:], r_t[:])
                        nc.vector.tensor_mul(
                            aoT[h2 * 64:h2 * 64 + 64, pr, q0:q1],
                            av_sb[0:64, :], R_t[:],
                        )
            if phase != "deep":
                # out-projection for this chunk's four n-tiles
                with tc.tile_pool(name=f"ps_o{rep}_{qc}", bufs=2,
                                  space="PSUM") as ps_op:
                    for nt in range(qc * 4, qc * 4 + 4):
                        o_sb = pout.tile([128, D], f32, tag="o", name="o_sb")
                        for dc in range(2):
                            ps_o = ps_op.tile([128, 512], f32, tag="o",
                                              name="ps_o")
                            for jt in range(4):
                                nc.tensor.matmul(
                                    ps_o[:],
                                    lhsT=aoT[:, jt, nt * 128:(nt + 1) * 128],
                                    rhs=woT_t[:, jt, dc * 512:(dc + 1) * 512],
                                    start=(jt == 0), stop=(jt == 3),
                                )
                            nc.vector.tensor_copy(
                                o_sb[:, dc * 512:(dc + 1) * 512], ps_o[:]
                            )
                        nc.sync.dma_start(
                            out[nt * 128:(nt + 1) * 128, :], o_sb[:])
        if phase == "deep":
            with (
                tc.tile_pool(name=f"ps_od{rep}", bufs=4, space="PSUM") as ps_op,
                tc.tile_pool(name=f"poutd{rep}", bufs=3) as poutd,
            ):
                for nt in range(16):
                    o_sb = poutd.tile([128, D], f32, tag="o", name="o_sb")
                    for dc in range(2):
                        ps_o = ps_op.tile([128, 512], f32, tag="o",
                                          name="ps_o")
                        for jt in range(4):
                            nc.tensor.matmul(
                                ps_o[:],
                                lhsT=aoT[:, jt, nt * 128:(nt + 1) * 128],
                                rhs=woT_t[:, jt, dc * 512:(dc + 1) * 512],
                                start=(jt == 0), stop=(jt == 3),
                            )
                        nc.vector.tensor_copy(
                            o_sb[:, dc * 512:(dc + 1) * 512], ps_o[:]
                        )
                    nc.sync.dma_start(out[nt * 128:(nt + 1) * 128, :], o_sb[:])


def _build_nc(reps=1, phase="all"):
    nc = bacc.Bacc(None, target_bir_lowering=False)
    h = {}
    for nm in ("xq", "xk", "xv"):
        h[nm] = nc.declare_dram_parameter(nm, [N, D], f32, isOutput=False)
    for nm in ("wqT", "wkT", "wvT"):
        h[nm] = nc.declare_dram_parameter(nm, [D, DH], f32, isOutput=False)
    h["woT"] = nc.declare_dram_parameter("woT", [DH, D], f32, isOutput=False)
    h["maskB"] = nc.declare_dram_parameter("maskB", [128, 896], f32,
                                           isOutput=False)
    h["ident"] = nc.declare_dram_parameter("ident", [128, 128], f32,
                                           isOutput=False)
    h["out"] = nc.declare_dram_parameter("out", [N, D], f32, isOutput=True)

    with tile.TileContext(nc) as tc:
        with (
            tc.tile_pool(name="consts", bufs=1) as consts,
            tc.tile_pool(name="qt", bufs=1) as qt_pool,
            tc.tile_pool(name="kt", bufs=1) as kt_pool,
            tc.tile_pool(name="vp", bufs=1) as vp_pool,
        ):
            h["ident_t"] = consts.tile([128, 128], f32, name="ident_t")
            nc.sync.dma_start(h["ident_t"][:], h["ident"][:])
            h["ident_r"] = consts.tile([128, 128], f32r, name="ident_r")
            nc.sync.dma_start(h["ident_r"][:], h["ident"][:].bitcast(f32r))
            h["maskB_t"] = consts.tile([128, 896], f32r, name="maskB_t")
            nc.sync.dma_start(h["maskB_t"][:], h["maskB"][:].bitcast(f32r))
            h["woT_t"] = consts.tile([128, 4, D], f32r, name="woT_t")
            nc.sync.dma_start(
                h["woT_t"][:],
                h["woT"][:].bitcast(f32r).rearrange("(o p) f -> p o f", p=128),
            )

            for rep in range(reps):
                h["QT"] = qt_pool.tile([128, 4, N], f32r, tag="QT", name="QT")
                h["KT"] = kt_pool.tile([128, 4, N], f32r, tag="KT", name="KT")
                h["Vp"] = vp_pool.tile([128, NT, 8, 65], f32r, tag="Vp",
                                       name="Vp")
                if phase == "attn":
                    nc.sync.dma_start(
                        h["QT"][:].rearrange("p a b -> p (a b)")
                        .rearrange("p (o f) -> p o f", o=8),
                        _junk3(h["xq"]),
                    )
                    nc.sync.dma_start(
                        h["KT"][:].rearrange("p a b -> p (a b)")
                        .rearrange("p (o f) -> p o f", o=8),
                        _junk3(h["xk"]),
                    )
                    nc.sync.dma_start(
                        h["Vp"][:].rearrange("p a b c -> p a (b c)"),
                        h["xv"][:].bitcast(f32r)
                        .rearrange("(o p) f -> p o f", p=128)[:, :, 0:520],
                    )
                else:
                    _phase1(nc, tc, rep, h, phase)

                if phase == "p1":
                    with tc.tile_pool(name=f"p1o{rep}", bufs=1) as p1o:
                        o_p1 = p1o.tile([128, D], f32, name="o_p1")
                        nc.vector.tensor_copy(
                            o_p1[:], h["QT"][:, 0, 0:1024].bitcast(f32))
                        nc.vector.tensor_copy(
                            o_p1[:, 0:1], h["Vp"][:, 0, 0, 0:1].bitcast(f32))
                        nc.vector.tensor_copy(
                            o_p1[:, 1:2], h["KT"][:, 0, 0:1].bitcast(f32))
                        nc.sync.dma_start(h["out"][0:128, :], o_p1[:])
                else:
                    _phase2(nc, tc, rep, h, phase)
    nc.compile()
    return nc


_NC = None


def _get_nc():
    global _NC
    if _NC is None:
        _NC = _build_nc()
    return _NC


def _make_in_maps(q, k, v, Wq, Wk, Wv, Wo):
    q = np.asarray(q, np.float32)
    k = np.asarray(k, np.float32)
    v = np.asarray(v, np.float32)
    Wq = np.asarray(Wq, np.float32)
    Wk = np.asarray(Wk, np.float32)
    Wv = np.asarray(Wv, np.float32)
    Wo = np.asarray(Wo, np.float32)

    kk = np.arange(128)[:, None]
    mm = np.arange(896)[None, :]
    maskB = np.where(kk <= mm - 384, 0.0, NEG).astype(np.float32)
    ident = np.eye(128, dtype=np.float32)

    in_maps = []
    for c in range(8):
        b, hh = divmod(c, 2)
        sl = slice(hh * DH, (hh + 1) * DH)
        in_maps.append({
            "xq": np.ascontiguousarray(q[b]),
            "xk": np.ascontiguousarray(k[b]),
            "xv": np.ascontiguousarray(v[b]),
            "wqT": np.ascontiguousarray(Wq[sl, :].T),
            "wkT": np.ascontiguousarray(Wk[sl, :].T),
            "wvT": np.ascontiguousarray(Wv[sl, :].T),
            "woT": np.ascontiguousarray(Wo[:, sl].T),
            "maskB": maskB,
            "ident": ident,
        })
    return in_maps


def kernel(q, k, v, Wq, Wk, Wv, Wo):
    nc = _get_nc()
    in_maps = _make_in_maps(q, k, v, Wq, Wk, Wv, Wo)
    res = run_bass_kernel_spmd(nc, in_maps, core_ids=list(range(8)))
    out = np.empty((B, N, D), np.float32)
    for b in range(B):
        out[b] = res.results[2 * b]["out"] + res.results[2 * b + 1]["out"]
    return out



# revision 4
# speedup vs baseline: 1.1204x; 1.1204x over previous
"""Multi-head causal attention (B=4, N=2048, D=1024, H=16) on 8 NeuronCores.

Sharding: data-parallel over batch (4) x tensor-parallel over heads (2 halves
of 8 heads each), Megatron-style.  Core c handles batch c//2 and head-half
c%2: Q/K/V projections restricted to its 512 output dims, full causal
attention for its 8 heads, and a partial output projection; the host sums the
two partials per batch.

v2 design (vs the f32r baseline):
  - x is transposed and bf16-converted on the HOST: xqT/xkT/xvT [1024, 2048]
    arrive transposed, so no PE transposes / PSUM round-trips on device.
  - everything PE-side is bf16 (same 1 cycle/row as f32r at >=256 free dim,
    but FWL weight loads + half DMA/SBUF).
  - fully chunk-interleaved: for each 512-row chunk sc: project Q/K/V(sc),
    then attention for q-chunk sc (causal => needs K/V chunks <= sc only),
    then out-projection of chunk sc.  Per-chunk tiles keep dependencies fine.
  - diagonal 512x512 block computed triangularly: k-tile jj covers only its
    valid q-cols (512/384/256/128), with a single [128,128] within-tile
    causal mask added via a 128-col identity matmul.
  - softmax row-sums via a ones-column appended to V (AV matmul M=65);
    normalization multiplied in on the way to aoT.
"""

import numpy as np
import ml_dtypes

import concourse.bass as bass
import concourse.bacc as bacc
import concourse.mybir as mybir
import concourse.tile as tile
from concourse.bass_utils import run_bass_kernel_spmd

B, N, D, H = 4, 2048, 1024, 16
HD = 64          # head dim
DH = 512         # per-core slice of D (8 heads)
NT = N // 128    # 16 seq tiles
CT = D // 128    # 8 feature tiles
NC = 4           # 512-row chunks
NEG = -1e30

f32 = mybir.dt.float32
bf16 = mybir.dt.bfloat16
EXP = mybir.ActivationFunctionType.Exp


def _proj_chunk(nc, h, pools, sc, name):
    """Project one input chunk: Q/K -> [dh, n] transposed; V -> Vp blocks."""
    xt = pools["xt"].tile([128, CT, 512], bf16, tag="xt")
    nc.sync.dma_start(
        xt[:],
        h["x" + name][:].rearrange("(o p) f -> p o f", p=128)
        [:, :, sc * 512:(sc + 1) * 512],
    )
    w_t = h["w_" + name]
    ps_pool = pools["mm"]
    if name in ("q", "k"):
        if name == "q":
            dst = pools["qt"].tile([128, 4, 512], bf16, tag="qt")
            h["QT"] = dst
        else:
            dst = pools["kt"].tile([128, 4, 512], bf16, tag=f"kt{sc}")
            h["KT"][sc] = dst
        for dt_ in range(4):
            ps = ps_pool.tile([128, 512], f32, tag="mm")
            for ct in range(CT):
                nc.tensor.matmul(
                    ps[:],
                    lhsT=w_t[:, ct, dt_ * 128:(dt_ + 1) * 128],
                    rhs=xt[:, ct, :],
                    start=(ct == 0), stop=(ct == CT - 1),
                )
            if dt_ % 2:
                nc.vector.tensor_copy(dst[:, dt_, :], ps[:])
            else:
                nc.scalar.copy(dst[:, dt_, :], ps[:])
    else:  # v
        dst = pools["vp"].tile([128, 4, 8, 65], bf16, tag=f"vp{sc}")
        h["Vp"][sc] = dst
        for st in range(4):
            ps = ps_pool.tile([128, 512], f32, tag="mm")
            for ct in range(CT):
                nc.tensor.matmul(
                    ps[:],
                    lhsT=xt[:, ct, st * 128:(st + 1) * 128],
                    rhs=w_t[:, ct, :],
                    start=(ct == 0), stop=(ct == CT - 1),
                )
            src = ps[:].rearrange("p (h d) -> p h d", h=8)
            if st % 2:
                nc.vector.tensor_copy(dst[:, st, :, 0:64], src)
            else:
                nc.scalar.copy(dst[:, st, :, 0:64], src)
        nc.vector.memset(dst[:, :, :, 64:65], 1.0)


def _attn_chunk(nc, h, pools, sc):
    """Causal attention for q-chunk sc over k chunks 0..sc, into aoT_c."""
    QT, KT, Vp = h["QT"], h["KT"], h["Vp"]
    mask_t, ident_t = h["mask_t"], h["ident_t"]
    aoT = pools["ao"].tile([128, 4, 512], bf16, tag="ao")
    h["aoT"] = aoT

    for pr in range(4):
        av = [
            pools["av"].tile([65, 512], f32, tag="av", name="av0"),
            pools["av"].tile([65, 512], f32, tag="av", name="av1"),
        ]
        nkt_full = sc * 4 + 1          # full-width k-tiles: 0 .. sc*4 (jj0)
        # groups of up to 2 full-width k-tiles, in k order
        groups = [
            list(range(kb, min(kb + 2, nkt_full)))
            for kb in range(0, nkt_full, 2)
        ]
        first_kt = groups[0][0]
        for kts in groups:
            s_ps = [
                pools["s"].tile([128, 2, 512], f32, tag="s", name="s0"),
                pools["s"].tile([128, 2, 512], f32, tag="s", name="s1"),
            ]
            for kti, kt in enumerate(kts):
                cc, lk = divmod(kt, 4)
                diag = kt == sc * 4    # jj0: needs mask on first 128 cols
                for h2 in (0, 1):
                    p0, p1 = h2 * 64, h2 * 64 + 64
                    nc.tensor.matmul(
                        s_ps[h2][:, kti, :],
                        lhsT=KT[cc][p0:p1, pr, lk * 128:(lk + 1) * 128],
                        rhs=QT[p0:p1, pr, :],
                        start=True, stop=not diag,
                    )
                    if diag:
                        nc.tensor.matmul(
                            s_ps[h2][:, kti, 0:128],
                            lhsT=ident_t[:],
                            rhs=mask_t[:],
                            start=False, stop=True,
                        )
            p_sb = [
                pools["p"].tile([128, 2, 512], bf16, tag="p", name="p0"),
                pools["p"].tile([128, 2, 512], bf16, tag="p", name="p1"),
            ]
            nkt = len(kts)
            for h2 in (0, 1):
                nc.scalar.activation(
                    p_sb[h2][:, :nkt, :], s_ps[h2][:, :nkt, :],
                    EXP, scale=0.125,
                )
            for kti, kt in enumerate(kts):
                cc, lk = divmod(kt, 4)
                for h2 in (0, 1):
                    hh = pr * 2 + h2
                    nc.tensor.matmul(
                        av[h2][:],
                        lhsT=Vp[cc][:, lk, hh, :],
                        rhs=p_sb[h2][:, kti, :],
                        start=(kt == first_kt), stop=False,
                    )

        # diagonal extras jj1..jj3, packed into one s_ps-shaped group:
        #   [:, 0, 0:384]  = jj1 (q cols 128:512)
        #   [:, 0, 384:512] = jj3 (q cols 384:512)
        #   [:, 1, 0:256]  = jj2 (q cols 256:512)
        s_ex = [
            pools["s"].tile([128, 2, 512], f32, tag="s", name="sx0"),
            pools["s"].tile([128, 2, 512], f32, tag="s", name="sx1"),
        ]
        ex_slices = (  # (bank, col0, width, jj, q0)
            (0, 0, 384, 1, 128),
            (0, 384, 128, 3, 384),
            (1, 0, 256, 2, 256),
        )
        for h2 in (0, 1):
            p0, p1 = h2 * 64, h2 * 64 + 64
            for bank, c0, w, jj, q0 in ex_slices:
                nc.tensor.matmul(
                    s_ex[h2][:, bank, c0:c0 + w],
                    lhsT=KT[sc][p0:p1, pr, jj * 128:(jj + 1) * 128],
                    rhs=QT[p0:p1, pr, q0:512],
                    start=True, stop=False,
                )
                nc.tensor.matmul(
                    s_ex[h2][:, bank, c0:c0 + 128],
                    lhsT=ident_t[:],
                    rhs=mask_t[:],
                    start=False, stop=True,
                )
        p_ex = [
            pools["p"].tile([128, 2, 512], bf16, tag="p", name="px0"),
            pools["p"].tile([128, 2, 512], bf16, tag="p", name="px1"),
        ]
        for h2 in (0, 1):
            nc.scalar.activation(
                p_ex[h2][:, 0, :], s_ex[h2][:, 0, :], EXP, scale=0.125)
            nc.scalar.activation(
                p_ex[h2][:, 1, 0:256], s_ex[h2][:, 1, 0:256], EXP, scale=0.125)
        for h2 in (0, 1):
            hh = pr * 2 + h2
            for i, (bank, c0, w, jj, q0) in enumerate(ex_slices):
                nc.tensor.matmul(
                    av[h2][:, q0:512],
                    lhsT=Vp[sc][:, jj, hh, :],
                    rhs=p_ex[h2][:, bank, c0:c0 + w],
                    start=False, stop=(i == len(ex_slices) - 1),
                )

        # normalize: aoT[head rows, :] = av[0:64] / av[64]
        for h2 in (0, 1):
            av_sb = pools["small"].tile([65, 512], f32, tag="avsb")
            nc.vector.tensor_copy(av_sb[:], av[h2][:])
            r_t = pools["small"].tile([1, 512], f32, tag="r")
            nc.vector.reciprocal(r_t[:], av_sb[64:65, :])
            R_t = pools["small"].tile([64, 512], f32, tag="R")
            nc.gpsimd.partition_broadcast(R_t[:], r_t[:])
            nc.vector.tensor_mul(
                aoT[h2 * 64:h2 * 64 + 64, pr, :], av_sb[0:64, :], R_t[:])


def _outproj_chunk(nc, h, pools, sc):
    aoT, woT_t, out = h["aoT"], h["woT_t"], h["out"]
    for ntl in range(4):
        o_sb = pools["o"].tile([128, D], f32, tag="o")
        for dc in range(2):
            ps_o = pools["mm"].tile([128, 512], f32, tag="mm")
            for jt in range(4):
                nc.tensor.matmul(
                    ps_o[:],
                    lhsT=aoT[:, jt, ntl * 128:(ntl + 1) * 128],
                    rhs=woT_t[:, jt, dc * 512:(dc + 1) * 512],
                    start=(jt == 0), stop=(jt == 3),
                )
            if dc:
                nc.vector.tensor_copy(o_sb[:, dc * 512:(dc + 1) * 512], ps_o[:])
            else:
                nc.scalar.copy(o_sb[:, dc * 512:(dc + 1) * 512], ps_o[:])
        nt = sc * 4 + ntl
        nc.sync.dma_start(out[nt * 128:(nt + 1) * 128, :], o_sb[:])


def _build_nc(reps=1, phase="all"):
    nc = bacc.Bacc(None, target_bir_lowering=False)
    h = {}
    for nm in ("xq", "xk", "xv"):
        h[nm] = nc.declare_dram_parameter(nm + "T", [D, N], bf16,
                                          isOutput=False)
    for nm in ("q", "k", "v"):
        h["wd_" + nm] = nc.declare_dram_parameter(
            "w" + nm, [D, DH], bf16, isOutput=False)
    h["woT"] = nc.declare_dram_parameter("woT", [DH, D], bf16, isOutput=False)
    h["mask"] = nc.declare_dram_parameter("mask128", [128, 128], bf16,
                                          isOutput=False)
    h["ident"] = nc.declare_dram_parameter("ident", [128, 128], bf16,
                                           isOutput=False)
    h["out"] = nc.declare_dram_parameter("out", [N, D], f32, isOutput=True)

    with tile.TileContext(nc) as tc:
        with (
            tc.tile_pool(name="consts", bufs=1) as consts,
            tc.tile_pool(name="kt", bufs=1) as kt_pool,
            tc.tile_pool(name="vp", bufs=1) as vp_pool,
            tc.tile_pool(name="qt", bufs=2) as qt_pool,
            tc.tile_pool(name="xt", bufs=3) as xt_pool,
            tc.tile_pool(name="ao", bufs=2) as ao_pool,
            tc.tile_pool(name="p", bufs=4) as p_pool,
            tc.tile_pool(name="small", bufs=4) as small_pool,
            tc.tile_pool(name="o", bufs=3) as o_pool,
            tc.tile_pool(name="ps_mm", bufs=2, space="PSUM") as mm_pool,
            tc.tile_pool(name="ps_s", bufs=2, space="PSUM") as s_pool,
            tc.tile_pool(name="ps_av", bufs=2, space="PSUM") as av_pool,
        ):
            for nm in ("q", "k", "v"):
                w_t = consts.tile([128, CT, DH], bf16, name=f"w_{nm}")
                nc.sync.dma_start(
                    w_t[:],
                    h["wd_" + nm][:].rearrange("(o p) f -> p o f", p=128),
                )
                h["w_" + nm] = w_t
            h["woT_t"] = consts.tile([128, 4, D], bf16, name="woT_t")
            nc.sync.dma_start(
                h["woT_t"][:],
                h["woT"][:].rearrange("(o p) f -> p o f", p=128),
            )
            h["mask_t"] = consts.tile([128, 128], bf16, name="mask_t")
            nc.sync.dma_start(h["mask_t"][:], h["mask"][:])
            h["ident_t"] = consts.tile([128, 128], bf16, name="ident_t")
            nc.sync.dma_start(h["ident_t"][:], h["ident"][:])

            pools = {
                "kt": kt_pool, "vp": vp_pool, "qt": qt_pool, "xt": xt_pool,
                "ao": ao_pool, "p": p_pool, "small": small_pool, "o": o_pool,
                "mm": mm_pool, "s": s_pool, "av": av_pool,
            }

            for rep in range(reps):
                h["KT"] = [None] * NC
                h["Vp"] = [None] * NC
                for sc in range(NC):
                    if phase == "attn":
                        # junk-fill instead of projecting
                        qt = qt_pool.tile([128, 4, 512], bf16, tag="qt")
                        nc.vector.memset(qt[:], 0.01)
                        h["QT"] = qt
                        kt = kt_pool.tile([128, 4, 512], bf16, tag=f"kt{sc}")
                        nc.vector.memset(kt[:], 0.01)
                        h["KT"][sc] = kt
                        vp = vp_pool.tile([128, 4, 8, 65], bf16, tag=f"vp{sc}")
                        nc.vector.memset(vp[:], 0.01)
                        h["Vp"][sc] = vp
                    else:
                        for nm in ("q", "k", "v"):
                            _proj_chunk(nc, h, pools, sc, nm)
                    if phase == "proj":
                        continue
                    _attn_chunk(nc, h, pools, sc)
                    _outproj_chunk(nc, h, pools, sc)
                if phase == "proj":
                    # keep outputs written so the NEFF has a sink
                    o_sb = o_pool.tile([128, D], f32, tag="o")
                    nc.vector.tensor_copy(
                        o_sb[:, 0:256], h["QT"][:, 0, :].bitcast(f32)[:, 0:256])
                    nc.sync.dma_start(h["out"][0:128, :], o_sb[:])
    nc.compile()
    return nc


_NC = None


def _get_nc():
    global _NC
    if _NC is None:
        _NC = _build_nc()
    return _NC


def _make_in_maps(q, k, v, Wq, Wk, Wv, Wo):
    q = np.asarray(q, np.float32)
    k = np.asarray(k, np.float32)
    v = np.asarray(v, np.float32)
    Wq = np.asarray(Wq, np.float32)
    Wk = np.asarray(Wk, np.float32)
    Wv = np.asarray(Wv, np.float32)
    Wo = np.asarray(Wo, np.float32)
    bf = ml_dtypes.bfloat16

    pp = np.arange(128)[:, None]
    jj = np.arange(128)[None, :]
    mask128 = np.where(pp > jj, NEG, 0.0).astype(bf)
    ident = np.eye(128, dtype=bf)

    xT = {}
    for b in range(B):
        xT[("q", b)] = np.ascontiguousarray(q[b].T).astype(bf)
        xT[("k", b)] = np.ascontiguousarray(k[b].T).astype(bf)
        xT[("v", b)] = np.ascontiguousarray(v[b].T).astype(bf)

    in_maps = []
    for c in range(8):
        b, hh = divmod(c, 2)
        sl = slice(hh * DH, (hh + 1) * DH)
        in_maps.append({
            "xqT": xT[("q", b)],
            "xkT": xT[("k", b)],
            "xvT": xT[("v", b)],
            "wq": np.ascontiguousarray(Wq[sl, :].T).astype(bf),
            "wk": np.ascontiguousarray(Wk[sl, :].T).astype(bf),
            "wv": np.ascontiguousarray(Wv[sl, :].T).astype(bf),
            "woT": np.ascontiguousarray(Wo[:, sl].T).astype(bf),
            "mask128": mask128,
            "ident": ident,
        })
    return in_maps


def kernel(q, k, v, Wq, Wk, Wv, Wo):
    nc = _get_nc()
    in_maps = _make_in_maps(q, k, v, Wq, Wk, Wv, Wo)
    res = run_bass_kernel_spmd(nc, in_maps, core_ids=list(range(8)))
    out = np.empty((B, N, D), np.float32)
    for b in range(B):
        out[b] = res.results[2 * b]["out"] + res.results[2 * b + 1]["out"]
    return out


# revision 6
# speedup vs baseline: 1.1897x; 1.0619x over previous
"""Multi-head causal attention (B=4, N=2048, D=1024, H=16) on 8 NeuronCores.

Sharding: data-parallel over batch (4) x tensor-parallel over heads (2 halves
of 8 heads each), Megatron-style.  Core c handles batch c//2 and head-half
c%2: Q/K/V projections restricted to its 512 output dims, full causal
attention for its 8 heads, and a partial output projection; the host sums the
two partials per batch.

v3 design:
  - x transposed + bf16-converted on the HOST (xqT/xkT/xvT [1024, 2048]):
    no PE transposes on device; all device matmuls bf16 (full PE rate, FWL
    weight loads, half DMA).
  - chunk-interleaved: project Q/K/V for 512-row chunk sc, attention for
    q-chunk sc (causal => K/V chunks <= sc), out-projection, repeat.
    Separate PSUM pools for projections (2 banks), out-proj (2), scores (2),
    AV accumulators (2) so chunk sc+1 projections overlap attention sc.
  - scores PSUM tile [128, 2, 512] holds BOTH head-halves of one k-tile
    (bank0=h0, bank1=h1): one exp instruction covers both heads.
  - diagonal 512-block computed triangularly (widths 512/384/256/128) with
    [128,128] identity-matmul mask adds; extras packed into 2 score tiles.
  - softmax row sums via ones-column in Vp (AV matmul M=65); normalization
    via reciprocal + partition-broadcast + multiply into aoT.
  - DMAs spread: x/weight loads on SP + ACT queues, output stores via
    gpsimd SWDGE.
"""

import numpy as np
import ml_dtypes

import concourse.bass as bass
import concourse.bacc as bacc
import concourse.mybir as mybir
import concourse.tile as tile
from concourse.bass_utils import run_bass_kernel_spmd

B, N, D, H = 4, 2048, 1024, 16
HD = 64          # head dim
DH = 512         # per-core slice of D (8 heads)
NT = N // 128    # 16 seq tiles
CT = D // 128    # 8 feature tiles
NC = 4           # 512-row chunks
NEG = -1e30

f32 = mybir.dt.float32
bf16 = mybir.dt.bfloat16
EXP = mybir.ActivationFunctionType.Exp


def _load_xt(nc, h, pools, sc, name):
    xt = pools["xt"].tile([128, CT, 512], bf16, tag=f"xt{name}", name="xt")
    nc.sync.dma_start(
        xt[:],
        h["x" + name][:].rearrange("(o p) f -> p o f", p=128)
        [:, :, sc * 512:(sc + 1) * 512],
    )
    return xt


def _proj_chunk(nc, h, pools, sc, name, xt):
    """Project one input chunk: Q/K -> [dh, n] transposed; V -> Vp blocks."""
    w_t = h["w_" + name]
    ps_pool = pools["mmp"]
    if name in ("q", "k"):
        if name == "q":
            dst = pools["qt"].tile([128, 4, 512], bf16, tag="qt", name="qt")
            h["QT"] = dst
        else:
            dst = pools["kt"].tile([128, 4, 512], bf16, tag=f"kt{sc}",
                                   name="kt")
            h["KT"][sc] = dst
        for dt_ in range(4):
            ps = ps_pool.tile([128, 512], f32, tag="mmp", name="psp")
            for ct in range(CT):
                nc.tensor.matmul(
                    ps[:],
                    lhsT=w_t[:, ct, dt_ * 128:(dt_ + 1) * 128],
                    rhs=xt[:, ct, :],
                    start=(ct == 0), stop=(ct == CT - 1),
                )
            nc.vector.tensor_copy(dst[:, dt_, :], ps[:])
    else:  # v
        dst = pools["vp"].tile([128, 4, 8, 65], bf16, tag=f"vp{sc}",
                               name="vp")
        h["Vp"][sc] = dst
        for st in range(4):
            ps = ps_pool.tile([128, 512], f32, tag="mmp", name="psp")
            for ct in range(CT):
                nc.tensor.matmul(
                    ps[:],
                    lhsT=xt[:, ct, st * 128:(st + 1) * 128],
                    rhs=w_t[:, ct, :],
                    start=(ct == 0), stop=(ct == CT - 1),
                )
            src = ps[:].rearrange("p (h d) -> p h d", h=8)
            nc.vector.tensor_copy(dst[:, st, :, 0:64], src)
        nc.vector.memset(dst[:, :, :, 64:65], 1.0)


def _attn_chunk(nc, h, pools, sc):
    """Causal attention for q-chunk sc over k chunks 0..sc, into aoT."""
    QT, KT, Vp = h["QT"], h["KT"], h["Vp"]
    mask_t, ident_t = h["mask_t"], h["ident_t"]
    aoT = pools["ao"].tile([128, 4, 512], bf16, tag="ao", name="ao")
    h["aoT"] = aoT

    for pr in range(4):
        av = [
            pools["av"].tile([65, 512], f32, tag="av", name="av0"),
            pools["av"].tile([65, 512], f32, tag="av", name="av1"),
        ]
        # full-width k-tiles 0 .. sc*4 (the last one, jj0, needs a mask on
        # its first 128 cols); one s tile per k-tile, bank per head-half.
        for kt in range(sc * 4 + 1):
            cc, lk = divmod(kt, 4)
            diag = kt == sc * 4
            s_ps = pools["s"].tile([128, 2, 512], f32, tag="s", name="s")
            for h2 in (0, 1):
                p0, p1 = h2 * 64, h2 * 64 + 64
                nc.tensor.matmul(
                    s_ps[:, h2, :],
                    lhsT=KT[cc][p0:p1, pr, lk * 128:(lk + 1) * 128],
                    rhs=QT[p0:p1, pr, :],
                    start=True, stop=not diag,
                )
                if diag:
                    nc.tensor.matmul(
                        s_ps[:, h2, 0:128],
                        lhsT=ident_t[:],
                        rhs=mask_t[:],
                        start=False, stop=True,
                    )
            p_sb = pools["p"].tile([128, 2, 512], bf16, tag="p", name="p")
            nc.scalar.activation(p_sb[:], s_ps[:], EXP, scale=0.125)
            for h2 in (0, 1):
                nc.tensor.matmul(
                    av[h2][:],
                    lhsT=Vp[cc][:, lk, pr * 2 + h2, :],
                    rhs=p_sb[:, h2, :],
                    start=(kt == 0), stop=False,
                )

        # diagonal extras jj1..jj3 (valid q-cols 384/256/128), two s tiles:
        #  ex1: bank h2 = [jj1 cols 0:384 | jj3 cols 384:512]
        #  ex2: bank 0  = [h0 jj2 cols 0:256 | h1 jj2 cols 256:512]
        ex1 = pools["s"].tile([128, 2, 512], f32, tag="s", name="ex1")
        for h2 in (0, 1):
            p0, p1 = h2 * 64, h2 * 64 + 64
            for c0, w, jj in ((0, 384, 1), (384, 128, 3)):
                q0 = jj * 128
                nc.tensor.matmul(
                    ex1[:, h2, c0:c0 + w],
                    lhsT=KT[sc][p0:p1, pr, jj * 128:(jj + 1) * 128],
                    rhs=QT[p0:p1, pr, q0:512],
                    start=True, stop=False,
                )
                nc.tensor.matmul(
                    ex1[:, h2, c0:c0 + 128],
                    lhsT=ident_t[:], rhs=mask_t[:],
                    start=False, stop=True,
                )
        ex2 = pools["s"].tile([128, 2, 512], f32, tag="s", name="ex2")
        for h2 in (0, 1):
            p0, p1 = h2 * 64, h2 * 64 + 64
            c0 = h2 * 256
            nc.tensor.matmul(
                ex2[:, 0, c0:c0 + 256],
                lhsT=KT[sc][p0:p1, pr, 256:384],
                rhs=QT[p0:p1, pr, 256:512],
                start=True, stop=False,
            )
            nc.tensor.matmul(
                ex2[:, 0, c0:c0 + 128],
                lhsT=ident_t[:], rhs=mask_t[:],
                start=False, stop=True,
            )
        px1 = pools["p"].tile([128, 2, 512], bf16, tag="p", name="px1")
        nc.scalar.activation(px1[:], ex1[:], EXP, scale=0.125)
        px2 = pools["p"].tile([128, 2, 512], bf16, tag="p", name="px2")
        nc.scalar.activation(px2[:, 0, :], ex2[:, 0, :], EXP, scale=0.125)
        for h2 in (0, 1):
            hh = pr * 2 + h2
            nc.tensor.matmul(
                av[h2][:, 128:512],
                lhsT=Vp[sc][:, 1, hh, :], rhs=px1[:, h2, 0:384],
                start=False, stop=False,
            )
            nc.tensor.matmul(
                av[h2][:, 256:512],
                lhsT=Vp[sc][:, 2, hh, :],
                rhs=px2[:, 0, h2 * 256:h2 * 256 + 256],
                start=False, stop=False,
            )
            nc.tensor.matmul(
                av[h2][:, 384:512],
                lhsT=Vp[sc][:, 3, hh, :], rhs=px1[:, h2, 384:512],
                start=False, stop=True,
            )

        # normalize: aoT[head rows, :] = av[0:64] / av[64]
        for h2 in (0, 1):
            av_sb = pools["small"].tile([65, 512], f32, tag="avsb",
                                        name="av_sb")
            nc.vector.tensor_copy(av_sb[:], av[h2][:])
            r_t = pools["small"].tile([1, 512], f32, tag="r", name="r_t")
            nc.vector.reciprocal(r_t[:], av_sb[64:65, :])
            R_t = pools["small"].tile([64, 512], f32, tag="R", name="R_t")
            nc.gpsimd.partition_broadcast(R_t[:], r_t[:])
            nc.gpsimd.tensor_mul(
                aoT[h2 * 64:h2 * 64 + 64, pr, :], av_sb[0:64, :], R_t[:])


def _outproj_chunk(nc, h, pools, sc):
    aoT, woT_t, out = h["aoT"], h["woT_t"], h["out"]
    for ntl in range(4):
        o_sb = pools["o"].tile([128, D], f32, tag="o", name="o_sb")
        ps_o = pools["s"].tile([128, 2, 512], f32, tag="s", name="ps_o")
        for dc in range(2):
            for jt in range(4):
                nc.tensor.matmul(
                    ps_o[:, dc, :],
                    lhsT=aoT[:, jt, ntl * 128:(ntl + 1) * 128],
                    rhs=woT_t[:, jt, dc * 512:(dc + 1) * 512],
                    start=(jt == 0), stop=(jt == 3),
                )
        nc.vector.tensor_copy(
            o_sb[:].rearrange("p (a b) -> p a b", a=2), ps_o[:])
        nt = sc * 4 + ntl
        nc.gpsimd.dma_start(out[nt * 128:(nt + 1) * 128, :], o_sb[:])


def _build_nc(reps=1, phase="all"):
    nc = bacc.Bacc(None, target_bir_lowering=False)
    h = {}
    for nm in ("xq", "xk", "xv"):
        h[nm] = nc.declare_dram_parameter(nm + "T", [D, N], bf16,
                                          isOutput=False)
    for nm in ("q", "k", "v"):
        h["wd_" + nm] = nc.declare_dram_parameter(
            "w" + nm, [D, DH], bf16, isOutput=False)
    h["woT"] = nc.declare_dram_parameter("woT", [DH, D], bf16, isOutput=False)
    h["mask"] = nc.declare_dram_parameter("mask128", [128, 128], bf16,
                                          isOutput=False)
    h["ident"] = nc.declare_dram_parameter("ident", [128, 128], bf16,
                                           isOutput=False)
    h["out"] = nc.declare_dram_parameter("out", [N, D], f32, isOutput=True)

    with tile.TileContext(nc) as tc:
        with (
            tc.tile_pool(name="consts", bufs=1) as consts,
            tc.tile_pool(name="kt", bufs=1) as kt_pool,
            tc.tile_pool(name="vp", bufs=1) as vp_pool,
            tc.tile_pool(name="qt", bufs=2) as qt_pool,
            tc.tile_pool(name="xt", bufs=2) as xt_pool,
            tc.tile_pool(name="ao", bufs=2) as ao_pool,
            tc.tile_pool(name="p", bufs=4) as p_pool,
            tc.tile_pool(name="small", bufs=4) as small_pool,
            tc.tile_pool(name="o", bufs=3) as o_pool,
            tc.tile_pool(name="ps_mmp", bufs=2, space="PSUM") as mmp_pool,
            tc.tile_pool(name="ps_s", bufs=2, space="PSUM") as s_pool,
            tc.tile_pool(name="ps_av", bufs=2, space="PSUM") as av_pool,
        ):
            # small consts via the ACT DMA queue (SP queue carries x inputs)
            h["mask_t"] = consts.tile([128, 128], bf16, name="mask_t")
            nc.scalar.dma_start(h["mask_t"][:], h["mask"][:])
            h["ident_t"] = consts.tile([128, 128], bf16, name="ident_t")
            nc.scalar.dma_start(h["ident_t"][:], h["ident"][:])
            for nm in ("q", "k", "v"):
                w_t = consts.tile([128, CT, DH], bf16, name=f"w_{nm}")
                eng = nc.sync if nm == "q" else nc.scalar
                eng.dma_start(
                    w_t[:],
                    h["wd_" + nm][:].rearrange("(o p) f -> p o f", p=128),
                )
                h["w_" + nm] = w_t
            h["woT_t"] = consts.tile([128, 4, D], bf16, name="woT_t")
            nc.scalar.dma_start(
                h["woT_t"][:],
                h["woT"][:].rearrange("(o p) f -> p o f", p=128),
            )

            pools = {
                "kt": kt_pool, "vp": vp_pool, "qt": qt_pool, "xt": xt_pool,
                "ao": ao_pool, "p": p_pool, "small": small_pool, "o": o_pool,
                "mmp": mmp_pool, "s": s_pool, "av": av_pool,
            }

            for rep in range(reps):
                h["KT"] = [None] * NC
                h["Vp"] = [None] * NC
                for sc in range(NC):
                    if phase == "attn":
                        qt = qt_pool.tile([128, 4, 512], bf16, tag="qt",
                                          name="qt")
                        nc.vector.memset(qt[:], 0.01)
                        h["QT"] = qt
                        kt = kt_pool.tile([128, 4, 512], bf16, tag=f"kt{sc}",
                                          name="kt")
                        nc.vector.memset(kt[:], 0.01)
                        h["KT"][sc] = kt
                        vp = vp_pool.tile([128, 4, 8, 65], bf16,
                                          tag=f"vp{sc}", name="vp")
                        nc.vector.memset(vp[:], 0.01)
                        h["Vp"][sc] = vp
                    else:
                        xts = {nm: _load_xt(nc, h, pools, sc, nm)
                               for nm in ("q", "k", "v")}
                        for nm in ("q", "k", "v"):
                            _proj_chunk(nc, h, pools, sc, nm, xts[nm])
                    if phase == "proj":
                        continue
                    _attn_chunk(nc, h, pools, sc)
                    _outproj_chunk(nc, h, pools, sc)
                if phase == "proj":
                    o_sb = o_pool.tile([128, D], f32, tag="o", name="o_sb")
                    nc.vector.tensor_copy(
                        o_sb[:, 0:256], h["QT"][:, 0, :].bitcast(f32)[:, 0:256])
                    nc.sync.dma_start(h["out"][0:128, :], o_sb[:])
    nc.compile()
    return nc


_NC = None


def _get_nc():
    global _NC
    if _NC is None:
        _NC = _build_nc()
    return _NC


def _make_in_maps(q, k, v, Wq, Wk, Wv, Wo):
    q = np.asarray(q, np.float32)
    k = np.asarray(k, np.float32)
    v = np.asarray(v, np.float32)
    Wq = np.asarray(Wq, np.float32)
    Wk = np.asarray(Wk, np.float32)
    Wv = np.asarray(Wv, np.float32)
    Wo = np.asarray(Wo, np.float32)
    bf = ml_dtypes.bfloat16

    pp = np.arange(128)[:, None]
    jj = np.arange(128)[None, :]
    mask128 = np.where(pp > jj, NEG, 0.0).astype(bf)
    ident = np.eye(128, dtype=bf)

    xT = {}
    for b in range(B):
        xT[("q", b)] = np.ascontiguousarray(q[b].T).astype(bf)
        xT[("k", b)] = np.ascontiguousarray(k[b].T).astype(bf)
        xT[("v", b)] = np.ascontiguousarray(v[b].T).astype(bf)

    in_maps = []
    for c in range(8):
        b, hh = divmod(c, 2)
        sl = slice(hh * DH, (hh + 1) * DH)
        in_maps.append({
            "xqT": xT[("q", b)],
            "xkT": xT[("k", b)],
            "xvT": xT[("v", b)],
            "wq": np.ascontiguousarray(Wq[sl, :].T).astype(bf),
            "wk": np.ascontiguousarray(Wk[sl, :].T).astype(bf),
            "wv": np.ascontiguousarray(Wv[sl, :].T).astype(bf),
            "woT": np.ascontiguousarray(Wo[:, sl].T).astype(bf),
            "mask128": mask128,
            "ident": ident,
        })
    return in_maps


def kernel(q, k, v, Wq, Wk, Wv, Wo):
    nc = _get_nc()
    in_maps = _make_in_maps(q, k, v, Wq, Wk, Wv, Wo)
    res = run_bass_kernel_spmd(nc, in_maps, core_ids=list(range(8)))
    out = np.empty((B, N, D), np.float32)
    for b in range(B):
        out[b] = res.results[2 * b]["out"] + res.results[2 * b + 1]["out"]
    return out


# revision 8
# speedup vs baseline: 2.0020x; 1.6827x over previous
"""Multi-head causal attention (B=4, N=2048, D=1024, H=16) on 8 NeuronCores.

Sharding: data-parallel over batch (4) x tensor-parallel over heads (2 halves
of 8 heads each), Megatron-style.  Core c handles batch c//2 and head-half
c%2: Q/K/V projections restricted to its 512 output dims, full causal
attention for its 8 heads, and a partial output projection; the host sums the
two partials per batch.

v3 design:
  - x transposed + bf16-converted on the HOST (xqT/xkT/xvT [1024, 2048]):
    no PE transposes on device; all device matmuls bf16 (full PE rate, FWL
    weight loads, half DMA).
  - chunk-interleaved: project Q/K/V for 512-row chunk sc, attention for
    q-chunk sc (causal => K/V chunks <= sc), out-projection, repeat.
    Separate PSUM pools for projections (2 banks), out-proj (2), scores (2),
    AV accumulators (2) so chunk sc+1 projections overlap attention sc.
  - scores PSUM tile [128, 2, 512] holds BOTH head-halves of one k-tile
    (bank0=h0, bank1=h1): one exp instruction covers both heads.
  - diagonal 512-block computed triangularly (widths 512/384/256/128) with
    [128,128] identity-matmul mask adds; extras packed into 2 score tiles.
  - softmax row sums via ones-column in Vp (AV matmul M=65); normalization
    via reciprocal + partition-broadcast + multiply into aoT.
  - DMAs spread: x/weight loads on SP + ACT queues, output stores via
    gpsimd SWDGE.
"""

import numpy as np
import ml_dtypes

import concourse.bass as bass
import concourse.bacc as bacc
import concourse.mybir as mybir
import concourse.tile as tile
from concourse.bass_utils import run_bass_kernel_spmd

B, N, D, H = 4, 2048, 1024, 16
HD = 64          # head dim
DH = 512         # per-core slice of D (8 heads)
NT = N // 128    # 16 seq tiles
CT = D // 128    # 8 feature tiles
NC = 4           # 512-row chunks
NEG = -1e30

f32 = mybir.dt.float32
bf16 = mybir.dt.bfloat16
EXP = mybir.ActivationFunctionType.Exp


def _load_xt(nc, h, pools, sc, name):
    xt = pools["xt"].tile([128, CT, 512], bf16, tag=f"xt{name}", name="xt")
    nc.sync.dma_start(
        xt[:],
        h["x" + name][:].rearrange("(o p) f -> p o f", p=128)
        [:, :, sc * 512:(sc + 1) * 512],
    )
    return xt


def _proj_chunk(nc, h, pools, sc, name, xt):
    """Project one input chunk: Q/K -> [dh, n] transposed; V -> Vp blocks."""
    w_t = h["w_" + name]
    ps_pool = pools["mmp"]
    if name in ("q", "k"):
        if name == "q":
            dst = pools["qt"].tile([128, 4, 512], bf16, tag="qt", name="qt")
            h["QT"] = dst
        else:
            dst = pools["kt"].tile([128, 4, 512], bf16, tag=f"kt{sc}",
                                   name="kt")
            h["KT"][sc] = dst
        for dt_ in range(4):
            ps = ps_pool.tile([128, 512], f32, tag="mmp", name="psp")
            for ct in range(CT):
                nc.tensor.matmul(
                    ps[:],
                    lhsT=w_t[:, ct, dt_ * 128:(dt_ + 1) * 128],
                    rhs=xt[:, ct, :],
                    start=(ct == 0), stop=(ct == CT - 1),
                )
            nc.vector.tensor_copy(dst[:, dt_, :], ps[:])
    else:  # v
        dst = pools["vp"].tile([128, 4, 8, 65], bf16, tag=f"vp{sc}",
                               name="vp")
        h["Vp"][sc] = dst
        for st in range(4):
            ps = ps_pool.tile([128, 512], f32, tag="mmp", name="psp")
            for ct in range(CT):
                nc.tensor.matmul(
                    ps[:],
                    lhsT=xt[:, ct, st * 128:(st + 1) * 128],
                    rhs=w_t[:, ct, :],
                    start=(ct == 0), stop=(ct == CT - 1),
                )
            src = ps[:].rearrange("p (h d) -> p h d", h=8)
            nc.vector.tensor_copy(dst[:, st, :, 0:64], src)
        nc.vector.memset(dst[:, :, :, 64:65], 1.0)


def _attn_chunk(nc, h, pools, sc, phase):
    """Causal attention for q-chunk sc over k chunks 0..sc, into aoT."""
    QT, KT, Vp = h["QT"], h["KT"], h["Vp"]
    mask_t, ident_t = h["mask_t"], h["ident_t"]
    aoT = pools["ao"].tile([128, 4, 512], bf16, tag="ao", name="ao")
    h["aoT"] = aoT

    for pr in range(4):
        av = None
        if phase != "noav":
            av = [
                pools["av"].tile([65, 512], f32, tag="av", name="av0"),
                pools["av"].tile([65, 512], f32, tag="av", name="av1"),
            ]
        # full-width k-tiles 0 .. sc*4 (the last one, jj0, needs a mask on
        # its first 128 cols); one s tile per k-tile, bank per head-half.
        for kt in range(sc * 4 + 1):
            cc, lk = divmod(kt, 4)
            diag = kt == sc * 4
            s_ps = pools["s"].tile([128, 2, 512], f32, tag="s", name="s")
            for h2 in (0, 1):
                p0, p1 = h2 * 64, h2 * 64 + 64
                nc.tensor.matmul(
                    s_ps[:, h2, :],
                    lhsT=KT[cc][p0:p1, pr, lk * 128:(lk + 1) * 128],
                    rhs=QT[p0:p1, pr, :],
                    start=True, stop=not diag,
                )
                if diag:
                    nc.tensor.matmul(
                        s_ps[:, h2, 0:128],
                        lhsT=ident_t[:],
                        rhs=mask_t[:],
                        start=False, stop=True,
                    )
            p_sb = pools["p"].tile([128, 2, 512], bf16, tag="p", name="p")
            if phase == "noexp":
                nc.scalar.activation(
                    p_sb[:, :, 0:1], s_ps[:, :, 0:1], EXP, scale=0.125)
            else:
                nc.scalar.activation(p_sb[:], s_ps[:], EXP, scale=0.125)
            if phase != "noav":
                for h2 in (0, 1):
                    nc.tensor.matmul(
                        av[h2][:],
                        lhsT=Vp[cc][:, lk, pr * 2 + h2, :],
                        rhs=p_sb[:, h2, :],
                        start=(kt == 0), stop=False,
                    )

        # diagonal extras jj1..jj3 (valid q-cols 384/256/128), two s tiles:
        #  ex1: bank h2 = [jj1 cols 0:384 | jj3 cols 384:512]
        #  ex2: bank 0  = [h0 jj2 cols 0:256 | h1 jj2 cols 256:512]
        ex1 = pools["s"].tile([128, 2, 512], f32, tag="s", name="ex1")
        for h2 in (0, 1):
            p0, p1 = h2 * 64, h2 * 64 + 64
            for c0, w, jj in ((0, 384, 1), (384, 128, 3)):
                q0 = jj * 128
                nc.tensor.matmul(
                    ex1[:, h2, c0:c0 + w],
                    lhsT=KT[sc][p0:p1, pr, jj * 128:(jj + 1) * 128],
                    rhs=QT[p0:p1, pr, q0:512],
                    start=True, stop=False,
                )
                nc.tensor.matmul(
                    ex1[:, h2, c0:c0 + 128],
                    lhsT=ident_t[:], rhs=mask_t[:],
                    start=False, stop=True,
                )
        ex2 = pools["s"].tile([128, 2, 512], f32, tag="s", name="ex2")
        for h2 in (0, 1):
            p0, p1 = h2 * 64, h2 * 64 + 64
            c0 = h2 * 256
            nc.tensor.matmul(
                ex2[:, 0, c0:c0 + 256],
                lhsT=KT[sc][p0:p1, pr, 256:384],
                rhs=QT[p0:p1, pr, 256:512],
                start=True, stop=False,
            )
            nc.tensor.matmul(
                ex2[:, 0, c0:c0 + 128],
                lhsT=ident_t[:], rhs=mask_t[:],
                start=False, stop=True,
            )
        px1 = pools["p"].tile([128, 2, 512], bf16, tag="p", name="px1")
        px2 = pools["p"].tile([128, 2, 512], bf16, tag="p", name="px2")
        if phase == "noexp":
            nc.scalar.activation(px1[:, :, 0:1], ex1[:, :, 0:1], EXP, scale=0.125)
            nc.scalar.activation(px2[:, 0, 0:1], ex2[:, 0, 0:1], EXP, scale=0.125)
        else:
            nc.scalar.activation(px1[:], ex1[:], EXP, scale=0.125)
            nc.scalar.activation(px2[:, 0, :], ex2[:, 0, :], EXP, scale=0.125)
        for h2 in (0, 1) if phase != "noav" else ():
            hh = pr * 2 + h2
            nc.tensor.matmul(
                av[h2][:, 128:512],
                lhsT=Vp[sc][:, 1, hh, :], rhs=px1[:, h2, 0:384],
                start=False, stop=False,
            )
            nc.tensor.matmul(
                av[h2][:, 256:512],
                lhsT=Vp[sc][:, 2, hh, :],
                rhs=px2[:, 0, h2 * 256:h2 * 256 + 256],
                start=False, stop=False,
            )
            nc.tensor.matmul(
                av[h2][:, 384:512],
                lhsT=Vp[sc][:, 3, hh, :], rhs=px1[:, h2, 384:512],
                start=False, stop=True,
            )

        # normalize: aoT[head rows, :] = av[0:64] / av[64]
        if phase == "noav":
            nc.vector.memset(aoT[:, pr, :], 0.01)
            continue
        for h2 in (0, 1):
            av_sb = pools["small"].tile([65, 512], f32, tag="avsb",
                                        name="av_sb")
            nc.vector.tensor_copy(av_sb[:], av[h2][:])
            r_t = pools["small"].tile([1, 512], f32, tag="r", name="r_t")
            nc.vector.reciprocal(r_t[:], av_sb[64:65, :])
            R_t = pools["small"].tile([64, 512], f32, tag="R", name="R_t")
            nc.gpsimd.partition_broadcast(R_t[:], r_t[:])
            nc.vector.tensor_mul(
                aoT[h2 * 64:h2 * 64 + 64, pr, :], av_sb[0:64, :], R_t[:])


def _outproj_chunk(nc, h, pools, sc):
    aoT, woT_t, out = h["aoT"], h["woT_t"], h["out"]
    for ntl in range(4):
        o_sb = pools["o"].tile([128, D], f32, tag="o", name="o_sb")
        ps_o = pools["s"].tile([128, 2, 512], f32, tag="s", name="ps_o")
        for dc in range(2):
            for jt in range(4):
                nc.tensor.matmul(
                    ps_o[:, dc, :],
                    lhsT=aoT[:, jt, ntl * 128:(ntl + 1) * 128],
                    rhs=woT_t[:, jt, dc * 512:(dc + 1) * 512],
                    start=(jt == 0), stop=(jt == 3),
                )
        nc.vector.tensor_copy(
            o_sb[:].rearrange("p (a b) -> p a b", a=2), ps_o[:])
        nt = sc * 4 + ntl
        nc.sync.dma_start(out[nt * 128:(nt + 1) * 128, :], o_sb[:])


def _build_nc(reps=1, phase="all"):
    nc = bacc.Bacc(None, target_bir_lowering=False)
    h = {}
    for nm in ("xq", "xk", "xv"):
        h[nm] = nc.declare_dram_parameter(nm + "T", [D, N], bf16,
                                          isOutput=False)
    for nm in ("q", "k", "v"):
        h["wd_" + nm] = nc.declare_dram_parameter(
            "w" + nm, [D, DH], bf16, isOutput=False)
    h["woT"] = nc.declare_dram_parameter("woT", [DH, D], bf16, isOutput=False)
    h["mask"] = nc.declare_dram_parameter("mask128", [128, 128], bf16,
                                          isOutput=False)
    h["ident"] = nc.declare_dram_parameter("ident", [128, 128], bf16,
                                           isOutput=False)
    h["out"] = nc.declare_dram_parameter("out", [N, D], f32, isOutput=True)

    with tile.TileContext(nc) as tc:
        with (
            tc.tile_pool(name="consts", bufs=1) as consts,
            tc.tile_pool(name="kt", bufs=1) as kt_pool,
            tc.tile_pool(name="vp", bufs=1) as vp_pool,
            tc.tile_pool(name="qt", bufs=2) as qt_pool,
            tc.tile_pool(name="xt", bufs=2) as xt_pool,
            tc.tile_pool(name="ao", bufs=2) as ao_pool,
            tc.tile_pool(name="p", bufs=4) as p_pool,
            tc.tile_pool(name="small", bufs=4) as small_pool,
            tc.tile_pool(name="o", bufs=3) as o_pool,
            tc.tile_pool(name="ps_mmp", bufs=2, space="PSUM") as mmp_pool,
            tc.tile_pool(name="ps_s", bufs=2, space="PSUM") as s_pool,
            tc.tile_pool(name="ps_av", bufs=2, space="PSUM") as av_pool,
        ):
            # small consts via the ACT DMA queue (SP queue carries x inputs)
            h["mask_t"] = consts.tile([128, 128], bf16, name="mask_t")
            nc.scalar.dma_start(h["mask_t"][:], h["mask"][:])
            h["ident_t"] = consts.tile([128, 128], bf16, name="ident_t")
            nc.scalar.dma_start(h["ident_t"][:], h["ident"][:])
            for nm in ("q", "k", "v"):
                w_t = consts.tile([128, CT, DH], bf16, name=f"w_{nm}")
                eng = nc.sync if nm == "q" else nc.scalar
                eng.dma_start(
                    w_t[:],
                    h["wd_" + nm][:].rearrange("(o p) f -> p o f", p=128),
                )
                h["w_" + nm] = w_t
            h["woT_t"] = consts.tile([128, 4, D], bf16, name="woT_t")
            nc.scalar.dma_start(
                h["woT_t"][:],
                h["woT"][:].rearrange("(o p) f -> p o f", p=128),
            )

            pools = {
                "kt": kt_pool, "vp": vp_pool, "qt": qt_pool, "xt": xt_pool,
                "ao": ao_pool, "p": p_pool, "small": small_pool, "o": o_pool,
                "mmp": mmp_pool, "s": s_pool, "av": av_pool,
            }

            for rep in range(reps):
                h["KT"] = [None] * NC
                h["Vp"] = [None] * NC
                for sc in range(NC):
                    if phase == "attn":
                        qt = qt_pool.tile([128, 4, 512], bf16, tag="qt",
                                          name="qt")
                        nc.vector.memset(qt[:], 0.01)
                        h["QT"] = qt
                        kt = kt_pool.tile([128, 4, 512], bf16, tag=f"kt{sc}",
                                          name="kt")
                        nc.vector.memset(kt[:], 0.01)
                        h["KT"][sc] = kt
                        vp = vp_pool.tile([128, 4, 8, 65], bf16,
                                          tag=f"vp{sc}", name="vp")
                        nc.vector.memset(vp[:], 0.01)
                        h["Vp"][sc] = vp
                    else:
                        xts = {nm: _load_xt(nc, h, pools, sc, nm)
                               for nm in ("q", "k", "v")}
                        for nm in ("q", "k", "v"):
                            _proj_chunk(nc, h, pools, sc, nm, xts[nm])
                    if phase == "proj":
                        continue
                    _attn_chunk(nc, h, pools, sc, phase)
                    _outproj_chunk(nc, h, pools, sc)
                if phase == "proj":
                    o_sb = o_pool.tile([128, D], f32, tag="o", name="o_sb")
                    nc.vector.tensor_copy(
                        o_sb[:, 0:256], h["QT"][:, 0, :].bitcast(f32)[:, 0:256])
                    nc.sync.dma_start(h["out"][0:128, :], o_sb[:])
    nc.compile()
    return nc


_NC = None


def _get_nc():
    global _NC
    if _NC is None:
        _NC = _build_nc()
    return _NC


def _make_in_maps(q, k, v, Wq, Wk, Wv, Wo):
    q = np.asarray(q, np.float32)
    k = np.asarray(k, np.float32)
    v = np.asarray(v, np.float32)
    Wq = np.asarray(Wq, np.float32)
    Wk = np.asarray(Wk, np.float32)
    Wv = np.asarray(Wv, np.float32)
    Wo = np.asarray(Wo, np.float32)
    bf = ml_dtypes.bfloat16

    pp = np.arange(128)[:, None]
    jj = np.arange(128)[None, :]
    mask128 = np.where(pp > jj, NEG, 0.0).astype(bf)
    ident = np.eye(128, dtype=bf)

    xT = {}
    for b in range(B):
        xT[("q", b)] = np.ascontiguousarray(q[b].T).astype(bf)
        xT[("k", b)] = np.ascontiguousarray(k[b].T).astype(bf)
        xT[("v", b)] = np.ascontiguousarray(v[b].T).astype(bf)

    in_maps = []
    for c in range(8):
        b, hh = divmod(c, 2)
        sl = slice(hh * DH, (hh + 1) * DH)
        in_maps.append({
            "xqT": xT[("q", b)],
            "xkT": xT[("k", b)],
            "xvT": xT[("v", b)],
            "wq": np.ascontiguousarray(Wq[sl, :].T).astype(bf),
            "wk": np.ascontiguousarray(Wk[sl, :].T).astype(bf),
            "wv": np.ascontiguousarray(Wv[sl, :].T).astype(bf),
            "woT": np.ascontiguousarray(Wo[:, sl].T).astype(bf),
            "mask128": mask128,
            "ident": ident,
        })
    return in_maps


def kernel(q, k, v, Wq, Wk, Wv, Wo):
    nc = _get_nc()
    in_maps = _make_in_maps(q, k, v, Wq, Wk, Wv, Wo)
    res = run_bass_kernel_spmd(nc, in_maps, core_ids=list(range(8)))
    out = np.empty((B, N, D), np.float32)
    for b in range(B):
        out[b] = res.results[2 * b]["out"] + res.results[2 * b + 1]["out"]
    return out
